# revision 22
# baseline (speedup 1.0000x reference)
"""CAWN attention+merge kernel for Trainium2, 8-core data parallel.

Math notes (vs the reference):
  - NS=1, so softmax over n of (score_q + score_k) == softmax(score_k):
    the per-query score is a constant shift and cancels. wq is never needed.
  - kh/vh are never materialized:
      score_k[b,n,h] = k[b,n,:] @ wkvT[:,h],  wkvT[d,h] = sum_e wk[h*DK+e,d]*wm[DK+e]
      out_fc[b]      = sum_h ctx[b,h,:] @ W2T[h],  W2T[h] = (fc_w[:,hblk] @ wv[hblk]).T
      ctx[b,h,:]     = attn[b,h,:] @ k[b]
  - LayerNorm gamma/beta are folded into m1_w/m1_b (LN output only feeds m1).
Sharding: batch 1024 -> 8 cores x 128. Each core processes 64 "tiles" of
[128 rows = 2 batches x 64 neighbors, 512 features].
"""

import numpy as np
import ml_dtypes

import concourse.bass as bass
import concourse.bacc as bacc
import concourse.tile as tile
import concourse.mybir as mybir
from concourse.bass_utils import run_bass_kernel_spmd

N_CORES = 8
B, NN, F = 1024, 64, 128
DM = 4 * F            # 512
H, DK = 2, 256
BL = B // N_CORES     # 128 batch per core
NT = BL // 2          # 64 tiles per core
NCH = DM // 128       # 4 d-chunks
LN_EPS = 1e-5

f32 = mybir.dt.float32
bf16 = mybir.dt.bfloat16
MUL = mybir.AluOpType.mult
ADD = mybir.AluOpType.add
AF = mybir.ActivationFunctionType

_CACHE = {}


def _build():
    nc = bacc.Bacc("TRN2", target_bir_lowering=False)

    # ---- DRAM tensors (per-core shapes) ----
    s_all = nc.dram_tensor("sall", [(NT // 16) * 128, 16 * DM], bf16, kind="ExternalInput")
    q_in = nc.dram_tensor("q", [BL, DM], f32, kind="ExternalInput")
    maskct_in = nc.dram_tensor("maskct", [1, NT * 128], bf16, kind="ExternalInput")
    ones2_in = nc.dram_tensor("ones2", [1, 2], bf16, kind="ExternalInput")
    wkvt_in = nc.dram_tensor("wkvt", [128, NCH * H], bf16, kind="ExternalInput")
    w2t_in = nc.dram_tensor("w2t", [128, H * NCH * DM], bf16, kind="ExternalInput")
    m1t_in = nc.dram_tensor("m1t", [128, 5 * F], f32, kind="ExternalInput")
    m2t_in = nc.dram_tensor("m2t", [128, F], f32, kind="ExternalInput")
    fcb_in = nc.dram_tensor("fcb", [1, DM], f32, kind="ExternalInput")
    m1b_in = nc.dram_tensor("m1b", [128, 1], f32, kind="ExternalInput")
    m2b_in = nc.dram_tensor("m2b", [128, 1], f32, kind="ExternalInput")
    eps_in = nc.dram_tensor("eps", [128, 1], f32, kind="ExternalInput")
    identb_in = nc.dram_tensor("identb", [128, 128], bf16, kind="ExternalInput")
    identf_in = nc.dram_tensor("identf", [128, 128], f32, kind="ExternalInput")
    blk4_in = nc.dram_tensor("blk4", [128, 4], f32, kind="ExternalInput")
    blkblk_in = nc.dram_tensor("blkblk", [128, 128], bf16, kind="ExternalInput")

    z_out = nc.dram_tensor("z", [BL, F], f32, kind="ExternalOutput")
    attn_out = nc.dram_tensor("attn", [H, BL, NN], f32, kind="ExternalOutput")

    with tile.TileContext(nc) as tc:
        with (
            tc.tile_pool(name="consts", bufs=1) as consts,
            tc.tile_pool(name="kb", bufs=2) as kbp,
            tc.tile_pool(name="kt", bufs=4) as ktp_sb,
            tc.tile_pool(name="small", bufs=4) as small,
            tc.tile_pool(name="ctxs", bufs=2) as ctxsp,
            tc.tile_pool(name="tail", bufs=1) as tailp,
            tc.tile_pool(name="ps", bufs=3, space="PSUM") as ps,
            tc.tile_pool(name="ps2", bufs=2, space="PSUM") as ps2,
            tc.tile_pool(name="psden", bufs=1, space="PSUM") as psden,
            tc.tile_pool(name="psacc", bufs=1, space="PSUM") as psacc,
        ):
            # ---- load constants ----
            def cload(tag, dram, shape, dtype):
                t = consts.tile(shape, dtype, tag=tag)
                nc.sync.dma_start(out=t[:], in_=dram[:])
                return t

            # critical-path constants first (needed by the streaming loop)
            identb = cload("identb", identb_in, [128, 128], bf16)
            wkvt = cload("wkvt", wkvt_in, [128, NCH * H], bf16)   # [p, c*2+h]
            maskct = cload("maskct", maskct_in, [1, NT * 128], bf16)
            ones2 = cload("ones2", ones2_in, [1, 2], bf16)
            blkblk = cload("blkblk", blkblk_in, [128, 128], bf16)
            blk4 = cload("blk4", blk4_in, [128, 4], f32)
            # late-use constants (g matmul + tail) loaded afterwards
            w2t = cload("w2t", w2t_in, [128, H * NCH * DM], bf16)
            q_sb = cload("q", q_in, [128, DM], f32)
            m1t = cload("m1t", m1t_in, [128, 5 * F], f32)
            m2t = cload("m2t", m2t_in, [128, F], f32)
            m1b = cload("m1b", m1b_in, [128, 1], f32)
            m2b = cload("m2b", m2b_in, [128, 1], f32)
            eps_sb = cload("eps", eps_in, [128, 1], f32)
            identf = cload("identf", identf_in, [128, 128], f32)

            fcb_rep = consts.tile([128, DM], f32, tag="fcb_rep")
            nc.sync.dma_start(
                out=fcb_rep[:],
                in_=bass.AP(tensor=fcb_in[:].tensor, offset=0,
                            ap=[[0, 128], [1, DM]]),
            )

            gp = psacc.tile([128, DM], f32, tag="g")

            ST = 16   # tiles per super-tile load
            KG = 8    # tiles per softmax batch group
            for half in range(2):
                ctxp = psacc.tile([128, NCH, 128], f32, tag="ctx")
                for jg in range(NT // 2 // KG):   # 4 groups of 8 tiles
                    tg0 = half * (NT // 2) + jg * KG
                    ti0 = jg * KG
                    # -- load K super-tile every ST tiles (pre-shuffled layout) --
                    if ti0 % ST == 0:
                        st_idx = (half * (NT // 2) + jg * KG) // ST
                        kbs = kbp.tile([128, ST, DM], bf16, tag="kbs")
                        nc.gpsimd.dma_start(
                            out=kbs[:],
                            in_=s_all[st_idx * 128:(st_idx + 1) * 128, :],
                        )
                        attn_acc = small.tile([128, ST, 2], f32, tag="attn_acc")

                    sc = ps2.tile([128, KG * H], f32, tag="b")
                    for jp in range(KG // 2):     # pairs of tiles
                        # transpose 8 chunks (2 tiles) into one psum bank
                        ktp = ps.tile([128, 2 * NCH, 128], bf16, tag="a")
                        kt = ktp_sb.tile([128, 2 * NCH, 128], bf16, tag="kt")
                        for u in range(2):
                            j = (ti0 % ST) + 2 * jp + u
                            kb = [kbs[:, j, c * 128:(c + 1) * 128]
                                  for c in range(NCH)]
                            for c in range(NCH):
                                nc.tensor.transpose(
                                    ktp[:, u * NCH + c, :], kb[c], identb[:])
                        # evacuate: ACT takes 3 chunks, DVE takes 5
                        nc.scalar.copy(kt[:, 0:3, :], ktp[:, 0:3, :])
                        nc.vector.tensor_copy(out=kt[:, 3:2 * NCH, :],
                                              in_=ktp[:, 3:2 * NCH, :])
                        # score (+ mask bias as a K=1 matmul)
                        for u in range(2):
                            j = 2 * jp + u
                            t = tg0 + j
                            for c in range(NCH):
                                nc.tensor.matmul(
                                    sc[:, 2 * j:2 * j + 2], kt[:, u * NCH + c, :],
                                    wkvt[:, c * H:(c + 1) * H],
                                    start=(c == 0), stop=False,
                                )
                            nc.tensor.matmul(
                                sc[:, 2 * j:2 * j + 2],
                                maskct[0:1, t * 128:(t + 1) * 128], ones2[0:1, :],
                                start=False, stop=True,
                            )

                    # -- batched softmax tail over KG tiles --
                    expb = small.tile([128, KG * H], bf16, tag="expb")
                    nc.scalar.activation(out=expb[:], in_=sc[:], func=AF.Exp,
                                         bias=0.0, scale=1.0)
                    den = psden.tile([128, KG * H], f32, tag="c")
                    nc.tensor.matmul(den[:], blkblk[:], expb[:], start=True, stop=True)
                    rden = small.tile([128, KG * H], f32, tag="rden")
                    nc.vector.reciprocal(rden[:], den[:])
                    nc.vector.tensor_tensor(
                        out=attn_acc[:, ti0 % ST:ti0 % ST + KG, :],
                        in0=expb[:], in1=rden[:], op=MUL,
                    )
                    # blocked attn for ctx matmuls: [128, (j,g,h)] bf16
                    attn4 = small.tile([128, KG * 4], bf16, tag="attn4")
                    af = attn_acc[:]
                    rep = bass.AP(
                        tensor=af.tensor, offset=af.offset + (ti0 % ST) * 2,
                        ap=[af.ap[0], [2, KG], [0, 2], [1, 2]],
                    )
                    blkr = bass.AP(
                        tensor=blk4[:].tensor, offset=blk4[:].offset,
                        ap=[blk4[:].ap[0], [0, KG], [1, 4]],
                    )
                    nc.vector.tensor_tensor(out=attn4[:], in0=rep, in1=blkr, op=MUL)
                    # ctx accumulation: ctxT[d, (tile,g,h)]
                    for j in range(KG):
                        ti = ti0 + j
                        kb = [kbs[:, ti % ST, c * 128:(c + 1) * 128]
                              for c in range(NCH)]
                        for c in range(NCH):
                            nc.tensor.matmul(
                                ctxp[:, c, 4 * ti:4 * ti + 4],
                                kb[c], attn4[:, 4 * j:4 * j + 4],
                                start=True, stop=True,
                            )

                    # -- attn output DMA once per super-tile (one per head) --
                    # dram elem addr = h*(BL*NN) + (2(t0+j)+g)*NN + n
                    if ti0 % ST == ST - KG:
                        t0 = tg0 - (ST - KG)
                        for h in range(H):
                            nc.sync.dma_start(
                                out=bass.AP(
                                    tensor=attn_out[:].tensor,
                                    offset=h * BL * NN + 2 * t0 * NN,
                                    ap=[[1, 128], [2 * NN, ST]],
                                ),
                                in_=attn_acc[:, :, h],
                            )

                # -- evacuate ctx half and run g matmuls --
                ctxs = ctxsp.tile([128, NCH, 128], bf16, tag="ctxs")
                nc.vector.tensor_copy(out=ctxs[:], in_=ctxp[:])
                for c in range(NCH):
                    for h in range(H):
                        lhs = bass.AP(
                            tensor=ctxs[:].tensor, offset=ctxs[:].offset + c * 128 + h,
                            ap=[ctxs[:].ap[0], [2, 64]],
                        )
                        nc.tensor.matmul(
                            gp[64 * half:64 * (half + 1), :],
                            lhs, w2t[:, (h * NCH + c) * DM:(h * NCH + c + 1) * DM],
                            start=(c == 0 and h == 0),
                            stop=(c == NCH - 1 and h == H - 1),
                        )

            # ---- tail: LeakyReLU(g+fcb) + q, LN, merge MLP ----
            g2 = tailp.tile([128, DM], f32)
            nc.vector.tensor_tensor(out=g2[:], in0=gp[:], in1=fcb_rep[:], op=ADD)
            lr = tailp.tile([128, DM], f32)
            nc.scalar.activation(out=lr[:], in_=g2[:], func=AF.Lrelu,
                                 bias=0.0, scale=1.0, alpha=0.2)
            xres = tailp.tile([128, DM], f32)
            nc.vector.tensor_tensor(out=xres[:], in0=lr[:], in1=q_sb[:], op=ADD)

            stats = tailp.tile([128, 6], f32)
            nc.vector.bn_stats(out=stats[:], in_=xres[:])
            mv = tailp.tile([128, 2], f32)
            nc.vector.bn_aggr(out=mv[:], in_=stats[:])
            sd = tailp.tile([128, 1], f32)
            nc.scalar.activation(out=sd[:], in_=mv[:, 1:2], func=AF.Sqrt,
                                 bias=eps_sb[:, 0:1], scale=1.0)
            rstd = tailp.tile([128, 1], f32)
            nc.vector.reciprocal(rstd[:], sd[:])
            negmr = tailp.tile([128, 1], f32)
            nc.vector.tensor_scalar(
                out=negmr[:], in0=mv[:, 0:1], scalar1=rstd[:, 0:1], scalar2=-1.0,
                op0=MUL, op1=MUL,
            )
            xn = tailp.tile([128, DM], f32)
            nc.scalar.activation(out=xn[:], in_=xres[:], func=AF.Identity,
                                 bias=negmr[:, 0:1], scale=rstd[:, 0:1])

            # x = [xn | src] transposed via PE (5 chunks of 128)
            xtp1 = ps.tile([128, 2, 128], f32, tag="a")
            xtp2 = ps2.tile([128, 2, 128], f32, tag="b")
            xtp3 = psden.tile([128, 1, 128], f32, tag="c")
            for c in range(2):
                nc.tensor.transpose(xtp1[:, c, :], xn[:, c * 128:(c + 1) * 128], identf[:])
            for c in range(2):
                nc.tensor.transpose(xtp2[:, c, :], xn[:, (2 + c) * 128:(3 + c) * 128], identf[:])
            nc.tensor.transpose(xtp3[:, 0, :], q_sb[:, 0:F], identf[:])
            xts = tailp.tile([128, 5, 128], f32)
            nc.vector.tensor_copy(out=xts[:, 0:2, :], in_=xtp1[:])
            nc.vector.tensor_copy(out=xts[:, 2:4, :], in_=xtp2[:])
            nc.scalar.copy(xts[:, 4, :], xtp3[:, 0, :])

            hdnp = ps.tile([128, F], f32, tag="a")
            for c in range(5):
                nc.tensor.matmul(hdnp[:], m1t[:, c * F:(c + 1) * F], xts[:, c, :],
                                 start=(c == 0), stop=(c == 4))
            hdn = tailp.tile([128, F], f32)
            nc.scalar.activation(out=hdn[:], in_=hdnp[:], func=AF.Relu,
                                 bias=m1b[:, 0:1], scale=1.0)
            ztp = ps2.tile([128, F], f32, tag="b")
            nc.tensor.matmul(ztp[:], m2t[:], hdn[:], start=True, stop=True)
            zts = tailp.tile([128, F], f32)
            nc.scalar.activation(out=zts[:], in_=ztp[:], func=AF.Identity,
                                 bias=m2b[:, 0:1], scale=1.0)
            zp = psden.tile([128, F], f32, tag="c")
            nc.tensor.transpose(zp[:], zts[:], identf[:])
            zs = tailp.tile([128, F], f32)
            nc.vector.tensor_copy(out=zs[:], in_=zp[:])
            nc.sync.dma_start(out=z_out[:], in_=zs[:])

    nc.compile()
    return nc


def _host_prep(inputs):
    src = np.asarray(inputs["src"], np.float32)
    src_t = np.asarray(inputs["src_t"], np.float32)
    src_p = np.asarray(inputs["src_p"], np.float32)
    mask = np.asarray(inputs["mask"])
    wk = np.asarray(inputs["wk"], np.float32)
    wv = np.asarray(inputs["wv"], np.float32)
    wm = np.asarray(inputs["wm"], np.float32)
    fc_w = np.asarray(inputs["fc_w"], np.float32)
    fc_b = np.asarray(inputs["fc_b"], np.float32)
    ln_g = np.asarray(inputs["ln_g"], np.float32)
    ln_b = np.asarray(inputs["ln_b"], np.float32)
    m1_w = np.asarray(inputs["m1_w"], np.float32)
    m1_b = np.asarray(inputs["m1_b"], np.float32)
    m2_w = np.asarray(inputs["m2_w"], np.float32)
    m2_b = np.asarray(inputs["m2_b"], np.float32)

    # q = concat(src, 0, src_t, src_p)  [B, DM]
    q = np.concatenate(
        [src[:, 0], np.zeros_like(src[:, 0]), src_t[:, 0], src_p[:, 0]], axis=-1
    ).astype(np.float32)

    # wkvT[d, h]
    wkvT = np.einsum("hed,e->dh", wk.reshape(H, DK, DM), wm[DK:]).astype(np.float32)
    # packed [128, c*2+h]
    wkvt_pack = np.ascontiguousarray(
        wkvT.reshape(NCH, 128, H).transpose(1, 0, 2).reshape(128, NCH * H)
    ).astype(ml_dtypes.bfloat16)

    # W2T[h] = (fc_w[:, hblk] @ wv[hblk]).T  -> [d, dd]
    w2t_pack = np.zeros((128, H * NCH * DM), np.float32)
    for h in range(H):
        w2 = fc_w[:, h * DK:(h + 1) * DK] @ wv[h * DK:(h + 1) * DK]   # [dd, d]
        w2T = w2.T  # [d, dd]
        for c in range(NCH):
            w2t_pack[:, (h * NCH + c) * DM:(h * NCH + c + 1) * DM] = \
                w2T[c * 128:(c + 1) * 128, :]
    w2t_pack = w2t_pack.astype(ml_dtypes.bfloat16)

    # LN folding into m1
    m1_w_eff = m1_w.copy()
    m1_w_eff[:, :DM] = m1_w[:, :DM] * ln_g[None, :]
    m1b_eff = m1_b + m1_w[:, :DM] @ ln_b
    m1T = m1_w_eff.T  # [640, 128]
    m1t_pack = np.ascontiguousarray(
        m1T.reshape(5, 128, F).transpose(1, 0, 2).reshape(128, 5 * F)
    ).astype(np.float32)
    m2t_pack = np.ascontiguousarray(m2_w.T).astype(np.float32)

    consts = dict(
        fcb=fc_b.reshape(1, DM).astype(np.float32),
        m1b=m1b_eff.reshape(128, 1).astype(np.float32),
        m2b=m2_b.reshape(128, 1).astype(np.float32),
        eps=np.full((128, 1), LN_EPS, np.float32),
        identb=np.eye(128, dtype=ml_dtypes.bfloat16),
        identf=np.eye(128, dtype=np.float32),
        wkvt=wkvt_pack, w2t=w2t_pack, m1t=m1t_pack, m2t=m2t_pack,
    )
    blk4 = np.zeros((128, 4), np.float32)
    blk4[:64, 0:2] = 1.0
    blk4[64:, 2:4] = 1.0
    consts["blk4"] = blk4
    consts["ones2"] = np.ones((1, 2), ml_dtypes.bfloat16)
    blkblk = np.zeros((128, 128), np.float32)
    blkblk[:64, :64] = 1.0
    blkblk[64:, 64:] = 1.0
    consts["blkblk"] = blkblk.astype(ml_dtypes.bfloat16)

    s_all = np.concatenate(
        [np.asarray(inputs["seq"], np.float32),
         np.asarray(inputs["seq_e"], np.float32),
         np.asarray(inputs["seq_t"], np.float32),
         np.asarray(inputs["seq_p"], np.float32)], axis=-1
    ).astype(ml_dtypes.bfloat16)  # [B, NN, DM]; device only consumes bf16 K
    # pre-shuffle to SBUF layout: [supertile, partition, tile-in-super, feat]
    # so each partition's super-tile block is one contiguous DRAM run

    in_maps = []
    for core in range(N_CORES):
        lo, hi = core * BL, (core + 1) * BL
        m = dict(consts)
        sc_ = s_all[lo:hi].reshape(NT // 16, 16, 128, DM).transpose(0, 2, 1, 3)
        m["sall"] = np.ascontiguousarray(sc_.reshape((NT // 16) * 128, 16 * DM))
        m["q"] = np.ascontiguousarray(q[lo:hi])
        mc = mask[lo:hi, 0, :]  # [BL, NN] bool
        # maskct[0, t*128 + p] = -1e10 * mask[2t + p//64, p%64]
        mct = (-1e10 * mc.reshape(NT, 128).astype(np.float32))
        m["maskct"] = mct.reshape(1, NT * 128).astype(ml_dtypes.bfloat16)
        in_maps.append(m)
    return in_maps


def kernel(**inputs):
    if "nc" not in _CACHE:
        _CACHE["nc"] = _build()
    nc = _CACHE["nc"]
    in_maps = _host_prep(inputs)
    res = run_bass_kernel_spmd(nc, in_maps, core_ids=list(range(N_CORES)))
    z = np.zeros((B, 1, F), np.float32)
    attn = np.zeros((H * B, 1, NN), np.float32)
    for core in range(N_CORES):
        r = res.results[core]
        lo = core * BL
        z[lo:lo + BL, 0, :] = r["z"]
        for h in range(H):
            attn[h * B + lo: h * B + lo + BL, 0, :] = r["attn"][h]
    return z, attn


# revision 23
# speedup vs baseline: 1.1117x; 1.1117x over previous
"""CAWN attention+merge kernel for Trainium2, 8-core data parallel.

Math notes (vs the reference):
  - NS=1, so softmax over n of (score_q + score_k) == softmax(score_k):
    the per-query score is a constant shift and cancels. wq is never needed.
  - kh/vh are never materialized:
      score_k[b,n,h] = k[b,n,:] @ wkvT[:,h],  wkvT[d,h] = sum_e wk[h*DK+e,d]*wm[DK+e]
      out_fc[b]      = sum_h ctx[b,h,:] @ W2T[h],  W2T[h] = (fc_w[:,hblk] @ wv[hblk]).T
      ctx[b,h,:]     = attn[b,h,:] @ k[b]
  - LayerNorm gamma/beta are folded into m1_w/m1_b (LN output only feeds m1).
Sharding: batch 1024 -> 8 cores x 128. Each core processes 64 "tiles" of
[128 rows = 2 batches x 64 neighbors, 512 features].
"""

import numpy as np
import ml_dtypes

import concourse.bass as bass
import concourse.bacc as bacc
import concourse.tile as tile
import concourse.mybir as mybir
from concourse.bass_utils import run_bass_kernel_spmd

N_CORES = 8
B, NN, F = 1024, 64, 128
DM = 4 * F            # 512
H, DK = 2, 256
BL = B // N_CORES     # 128 batch per core
NT = BL // 2          # 64 tiles per core
NCH = DM // 128       # 4 d-chunks
LN_EPS = 1e-5

f32 = mybir.dt.float32
bf16 = mybir.dt.bfloat16
MUL = mybir.AluOpType.mult
ADD = mybir.AluOpType.add
AF = mybir.ActivationFunctionType

_CACHE = {}


def _build():
    nc = bacc.Bacc("TRN2", target_bir_lowering=False)

    # ---- DRAM tensors (per-core shapes) ----
    s_all = nc.dram_tensor("sall", [(NT // 16) * 128, 16 * DM], bf16, kind="ExternalInput")
    q_in = nc.dram_tensor("q", [BL, DM], f32, kind="ExternalInput")
    maskct_in = nc.dram_tensor("maskct", [1, NT * 128], bf16, kind="ExternalInput")
    ones2_in = nc.dram_tensor("ones2", [1, 2], bf16, kind="ExternalInput")
    wkvt_in = nc.dram_tensor("wkvt", [128, NCH * H], bf16, kind="ExternalInput")
    w2t_in = nc.dram_tensor("w2t", [128, H * NCH * DM], bf16, kind="ExternalInput")
    m1t_in = nc.dram_tensor("m1t", [128, 5 * F], f32, kind="ExternalInput")
    m2t_in = nc.dram_tensor("m2t", [128, F], f32, kind="ExternalInput")
    fcb_in = nc.dram_tensor("fcb", [1, DM], f32, kind="ExternalInput")
    m1b_in = nc.dram_tensor("m1b", [128, 1], f32, kind="ExternalInput")
    m2b_in = nc.dram_tensor("m2b", [128, 1], f32, kind="ExternalInput")
    eps_in = nc.dram_tensor("eps", [128, 1], f32, kind="ExternalInput")
    identb_in = nc.dram_tensor("identb", [128, 128], bf16, kind="ExternalInput")
    identf_in = nc.dram_tensor("identf", [128, 128], f32, kind="ExternalInput")
    blk4_in = nc.dram_tensor("blk4", [128, 4], f32, kind="ExternalInput")
    blkblk_in = nc.dram_tensor("blkblk", [128, 128], bf16, kind="ExternalInput")

    z_out = nc.dram_tensor("z", [BL, F], f32, kind="ExternalOutput")
    attn_out = nc.dram_tensor("attn", [H, BL, NN], f32, kind="ExternalOutput")

    with tile.TileContext(nc) as tc:
        with (
            tc.tile_pool(name="consts", bufs=1) as consts,
            tc.tile_pool(name="kb", bufs=2) as kbp,
            tc.tile_pool(name="kt", bufs=4) as ktp_sb,
            tc.tile_pool(name="small", bufs=4) as small,
            tc.tile_pool(name="ctxs", bufs=2) as ctxsp,
            tc.tile_pool(name="tail", bufs=1) as tailp,
            tc.tile_pool(name="ps", bufs=3, space="PSUM") as ps,
            tc.tile_pool(name="ps2", bufs=2, space="PSUM") as ps2,
            tc.tile_pool(name="psacc", bufs=1, space="PSUM") as psacc,
        ):
            # ---- load constants ----
            def cload(tag, dram, shape, dtype):
                t = consts.tile(shape, dtype, tag=tag)
                nc.sync.dma_start(out=t[:], in_=dram[:])
                return t

            # critical-path constants first (needed by the streaming loop)
            identb = cload("identb", identb_in, [128, 128], bf16)
            wkvt = cload("wkvt", wkvt_in, [128, NCH * H], bf16)   # [p, c*2+h]
            maskct = cload("maskct", maskct_in, [1, NT * 128], bf16)
            ones2 = cload("ones2", ones2_in, [1, 2], bf16)
            blkblk = cload("blkblk", blkblk_in, [128, 128], bf16)
            blk4 = cload("blk4", blk4_in, [128, 4], f32)
            # late-use constants (g matmul + tail) loaded afterwards
            w2t = cload("w2t", w2t_in, [128, H * NCH * DM], bf16)
            q_sb = cload("q", q_in, [128, DM], f32)
            m1t = cload("m1t", m1t_in, [128, 5 * F], f32)
            m2t = cload("m2t", m2t_in, [128, F], f32)
            m1b = cload("m1b", m1b_in, [128, 1], f32)
            m2b = cload("m2b", m2b_in, [128, 1], f32)
            eps_sb = cload("eps", eps_in, [128, 1], f32)
            identf = cload("identf", identf_in, [128, 128], f32)

            fcb_rep = consts.tile([128, DM], f32, tag="fcb_rep")
            nc.sync.dma_start(
                out=fcb_rep[:],
                in_=bass.AP(tensor=fcb_in[:].tensor, offset=0,
                            ap=[[0, 128], [1, DM]]),
            )

            gp = psacc.tile([128, DM], f32, tag="g")

            ST = 16   # tiles per super-tile load
            KG = 8    # tiles per softmax batch group
            for half in range(2):
                ctxp = ps2.tile([128, NCH, 128], f32, tag="ctx")
                for jg in range(NT // 2 // KG):   # 4 groups of 8 tiles
                    tg0 = half * (NT // 2) + jg * KG
                    ti0 = jg * KG
                    # -- load K super-tile every ST tiles (pre-shuffled layout,
                    #    4 quarter-DMAs so consumers start early) --
                    if ti0 % ST == 0:
                        st_idx = (half * (NT // 2) + jg * KG) // ST
                        kbs = kbp.tile([128, ST, DM], bf16, tag="kbs")
                        for qq in range(4):
                            nc.gpsimd.dma_start(
                                out=kbs[:, qq * 4:(qq + 1) * 4, :],
                                in_=s_all[st_idx * 128:(st_idx + 1) * 128,
                                          qq * 4 * DM:(qq + 1) * 4 * DM],
                            )
                        attn_acc = small.tile([128, ST, 2], f32, tag="attn_acc")

                    scb = ps2.tile([128, 2 * KG * H], f32, tag="b")
                    sc = scb[:, 0:KG * H]
                    for jp in range(KG // 2):     # pairs of tiles
                        # transpose 8 chunks (2 tiles) into one psum bank
                        ktp = ps.tile([128, 2 * NCH, 128], bf16, tag="a")
                        kt = ktp_sb.tile([128, 2 * NCH, 128], bf16, tag="kt")
                        for u in range(2):
                            j = (ti0 % ST) + 2 * jp + u
                            kb = [kbs[:, j, c * 128:(c + 1) * 128]
                                  for c in range(NCH)]
                            for c in range(NCH):
                                nc.tensor.transpose(
                                    ktp[:, u * NCH + c, :], kb[c], identb[:])
                        # evacuate: ACT takes 3 chunks, DVE takes 5
                        nc.scalar.copy(kt[:, 0:3, :], ktp[:, 0:3, :])
                        nc.vector.tensor_copy(out=kt[:, 3:2 * NCH, :],
                                              in_=ktp[:, 3:2 * NCH, :])
                        # score (+ mask bias as a K=1 matmul)
                        for u in range(2):
                            j = 2 * jp + u
                            t = tg0 + j
                            for c in range(NCH):
                                nc.tensor.matmul(
                                    sc[:, 2 * j:2 * j + 2], kt[:, u * NCH + c, :],
                                    wkvt[:, c * H:(c + 1) * H],
                                    start=(c == 0), stop=False,
                                )
                            nc.tensor.matmul(
                                sc[:, 2 * j:2 * j + 2],
                                maskct[0:1, t * 128:(t + 1) * 128], ones2[0:1, :],
                                start=False, stop=True,
                            )

                    # -- batched softmax tail over KG tiles --
                    expb = small.tile([128, KG * H], bf16, tag="expb")
                    nc.scalar.activation(out=expb[:], in_=sc, func=AF.Exp,
                                         bias=0.0, scale=1.0)
                    den = scb[:, KG * H:2 * KG * H]
                    nc.tensor.matmul(den, blkblk[:], expb[:], start=True, stop=True)
                    rden = small.tile([128, KG * H], f32, tag="rden")
                    nc.vector.reciprocal(rden[:], den)
                    nc.vector.tensor_tensor(
                        out=attn_acc[:, ti0 % ST:ti0 % ST + KG, :],
                        in0=expb[:], in1=rden[:], op=MUL,
                    )
                    # blocked attn for ctx matmuls: [128, (j,g,h)] bf16
                    attn4 = small.tile([128, KG * 4], bf16, tag="attn4")
                    af = attn_acc[:]
                    rep = bass.AP(
                        tensor=af.tensor, offset=af.offset + (ti0 % ST) * 2,
                        ap=[af.ap[0], [2, KG], [0, 2], [1, 2]],
                    )
                    blkr = bass.AP(
                        tensor=blk4[:].tensor, offset=blk4[:].offset,
                        ap=[blk4[:].ap[0], [0, KG], [1, 4]],
                    )
                    nc.vector.tensor_tensor(out=attn4[:], in0=rep, in1=blkr, op=MUL)
                    # ctx accumulation: ctxT[d, (tile,g,h)]
                    for j in range(KG):
                        ti = ti0 + j
                        kb = [kbs[:, ti % ST, c * 128:(c + 1) * 128]
                              for c in range(NCH)]
                        for c in range(NCH):
                            nc.tensor.matmul(
                                ctxp[:, c, 4 * ti:4 * ti + 4],
                                kb[c], attn4[:, 4 * j:4 * j + 4],
                                start=True, stop=True,
                            )

                    # -- attn output DMA once per super-tile (one per head) --
                    # dram elem addr = h*(BL*NN) + (2(t0+j)+g)*NN + n
                    if ti0 % ST == ST - KG:
                        t0 = tg0 - (ST - KG)
                        for h in range(H):
                            nc.sync.dma_start(
                                out=bass.AP(
                                    tensor=attn_out[:].tensor,
                                    offset=h * BL * NN + 2 * t0 * NN,
                                    ap=[[1, 128], [2 * NN, ST]],
                                ),
                                in_=attn_acc[:, :, h],
                            )

                # -- evacuate ctx half and run g matmuls --
                ctxs = ctxsp.tile([128, NCH, 128], bf16, tag="ctxs")
                nc.vector.tensor_copy(out=ctxs[:], in_=ctxp[:])
                for c in range(NCH):
                    for h in range(H):
                        lhs = bass.AP(
                            tensor=ctxs[:].tensor, offset=ctxs[:].offset + c * 128 + h,
                            ap=[ctxs[:].ap[0], [2, 64]],
                        )
                        nc.tensor.matmul(
                            gp[64 * half:64 * (half + 1), :],
                            lhs, w2t[:, (h * NCH + c) * DM:(h * NCH + c + 1) * DM],
                            start=(c == 0 and h == 0),
                            stop=(c == NCH - 1 and h == H - 1),
                        )

            # ---- tail: LeakyReLU(g+fcb) + q, LN, merge MLP ----
            g2 = tailp.tile([128, DM], f32)
            nc.vector.tensor_tensor(out=g2[:], in0=gp[:], in1=fcb_rep[:], op=ADD)
            lr = tailp.tile([128, DM], f32)
            nc.scalar.activation(out=lr[:], in_=g2[:], func=AF.Lrelu,
                                 bias=0.0, scale=1.0, alpha=0.2)
            xres = tailp.tile([128, DM], f32)
            nc.vector.tensor_tensor(out=xres[:], in0=lr[:], in1=q_sb[:], op=ADD)

            stats = tailp.tile([128, 6], f32)
            nc.vector.bn_stats(out=stats[:], in_=xres[:])
            mv = tailp.tile([128, 2], f32)
            nc.vector.bn_aggr(out=mv[:], in_=stats[:])
            sd = tailp.tile([128, 1], f32)
            nc.scalar.activation(out=sd[:], in_=mv[:, 1:2], func=AF.Sqrt,
                                 bias=eps_sb[:, 0:1], scale=1.0)
            rstd = tailp.tile([128, 1], f32)
            nc.vector.reciprocal(rstd[:], sd[:])
            negmr = tailp.tile([128, 1], f32)
            nc.vector.tensor_scalar(
                out=negmr[:], in0=mv[:, 0:1], scalar1=rstd[:, 0:1], scalar2=-1.0,
                op0=MUL, op1=MUL,
            )
            xn = tailp.tile([128, DM], f32)
            nc.scalar.activation(out=xn[:], in_=xres[:], func=AF.Identity,
                                 bias=negmr[:, 0:1], scale=rstd[:, 0:1])

            # x = [xn | src] transposed via PE (5 chunks of 128)
            xtp1 = ps.tile([128, 2, 128], f32, tag="a")
            xtp2 = ps2.tile([128, 2, 128], f32, tag="b")
            xtp3 = ps2.tile([128, 1, 128], f32, tag="ctx")
            for c in range(2):
                nc.tensor.transpose(xtp1[:, c, :], xn[:, c * 128:(c + 1) * 128], identf[:])
            for c in range(2):
                nc.tensor.transpose(xtp2[:, c, :], xn[:, (2 + c) * 128:(3 + c) * 128], identf[:])
            nc.tensor.transpose(xtp3[:, 0, :], q_sb[:, 0:F], identf[:])
            xts = tailp.tile([128, 5, 128], f32)
            nc.vector.tensor_copy(out=xts[:, 0:2, :], in_=xtp1[:])
            nc.vector.tensor_copy(out=xts[:, 2:4, :], in_=xtp2[:])
            nc.scalar.copy(xts[:, 4, :], xtp3[:, 0, :])

            hdnp = ps.tile([128, F], f32, tag="a")
            for c in range(5):
                nc.tensor.matmul(hdnp[:], m1t[:, c * F:(c + 1) * F], xts[:, c, :],
                                 start=(c == 0), stop=(c == 4))
            hdn = tailp.tile([128, F], f32)
            nc.scalar.activation(out=hdn[:], in_=hdnp[:], func=AF.Relu,
                                 bias=m1b[:, 0:1], scale=1.0)
            ztp = ps2.tile([128, F], f32, tag="b")
            nc.tensor.matmul(ztp[:], m2t[:], hdn[:], start=True, stop=True)
            zts = tailp.tile([128, F], f32)
            nc.scalar.activation(out=zts[:], in_=ztp[:], func=AF.Identity,
                                 bias=m2b[:, 0:1], scale=1.0)
            zp = ps2.tile([128, F], f32, tag="ctx")
            nc.tensor.transpose(zp[:], zts[:], identf[:])
            zs = tailp.tile([128, F], f32)
            nc.vector.tensor_copy(out=zs[:], in_=zp[:])
            nc.sync.dma_start(out=z_out[:], in_=zs[:])

    nc.compile()
    return nc


def _host_prep(inputs):
    src = np.asarray(inputs["src"], np.float32)
    src_t = np.asarray(inputs["src_t"], np.float32)
    src_p = np.asarray(inputs["src_p"], np.float32)
    mask = np.asarray(inputs["mask"])
    wk = np.asarray(inputs["wk"], np.float32)
    wv = np.asarray(inputs["wv"], np.float32)
    wm = np.asarray(inputs["wm"], np.float32)
    fc_w = np.asarray(inputs["fc_w"], np.float32)
    fc_b = np.asarray(inputs["fc_b"], np.float32)
    ln_g = np.asarray(inputs["ln_g"], np.float32)
    ln_b = np.asarray(inputs["ln_b"], np.float32)
    m1_w = np.asarray(inputs["m1_w"], np.float32)
    m1_b = np.asarray(inputs["m1_b"], np.float32)
    m2_w = np.asarray(inputs["m2_w"], np.float32)
    m2_b = np.asarray(inputs["m2_b"], np.float32)

    # q = concat(src, 0, src_t, src_p)  [B, DM]
    q = np.concatenate(
        [src[:, 0], np.zeros_like(src[:, 0]), src_t[:, 0], src_p[:, 0]], axis=-1
    ).astype(np.float32)

    # wkvT[d, h]
    wkvT = np.einsum("hed,e->dh", wk.reshape(H, DK, DM), wm[DK:]).astype(np.float32)
    # packed [128, c*2+h]
    wkvt_pack = np.ascontiguousarray(
        wkvT.reshape(NCH, 128, H).transpose(1, 0, 2).reshape(128, NCH * H)
    ).astype(ml_dtypes.bfloat16)

    # W2T[h] = (fc_w[:, hblk] @ wv[hblk]).T  -> [d, dd]
    w2t_pack = np.zeros((128, H * NCH * DM), np.float32)
    for h in range(H):
        w2 = fc_w[:, h * DK:(h + 1) * DK] @ wv[h * DK:(h + 1) * DK]   # [dd, d]
        w2T = w2.T  # [d, dd]
        for c in range(NCH):
            w2t_pack[:, (h * NCH + c) * DM:(h * NCH + c + 1) * DM] = \
                w2T[c * 128:(c + 1) * 128, :]
    w2t_pack = w2t_pack.astype(ml_dtypes.bfloat16)

    # LN folding into m1
    m1_w_eff = m1_w.copy()
    m1_w_eff[:, :DM] = m1_w[:, :DM] * ln_g[None, :]
    m1b_eff = m1_b + m1_w[:, :DM] @ ln_b
    m1T = m1_w_eff.T  # [640, 128]
    m1t_pack = np.ascontiguousarray(
        m1T.reshape(5, 128, F).transpose(1, 0, 2).reshape(128, 5 * F)
    ).astype(np.float32)
    m2t_pack = np.ascontiguousarray(m2_w.T).astype(np.float32)

    consts = dict(
        fcb=fc_b.reshape(1, DM).astype(np.float32),
        m1b=m1b_eff.reshape(128, 1).astype(np.float32),
        m2b=m2_b.reshape(128, 1).astype(np.float32),
        eps=np.full((128, 1), LN_EPS, np.float32),
        identb=np.eye(128, dtype=ml_dtypes.bfloat16),
        identf=np.eye(128, dtype=np.float32),
        wkvt=wkvt_pack, w2t=w2t_pack, m1t=m1t_pack, m2t=m2t_pack,
    )
    blk4 = np.zeros((128, 4), np.float32)
    blk4[:64, 0:2] = 1.0
    blk4[64:, 2:4] = 1.0
    consts["blk4"] = blk4
    consts["ones2"] = np.ones((1, 2), ml_dtypes.bfloat16)
    blkblk = np.zeros((128, 128), np.float32)
    blkblk[:64, :64] = 1.0
    blkblk[64:, 64:] = 1.0
    consts["blkblk"] = blkblk.astype(ml_dtypes.bfloat16)

    s_all = np.concatenate(
        [np.asarray(inputs["seq"], np.float32),
         np.asarray(inputs["seq_e"], np.float32),
         np.asarray(inputs["seq_t"], np.float32),
         np.asarray(inputs["seq_p"], np.float32)], axis=-1
    ).astype(ml_dtypes.bfloat16)  # [B, NN, DM]; device only consumes bf16 K
    # pre-shuffle to SBUF layout: [supertile, partition, tile-in-super, feat]
    # so each partition's super-tile block is one contiguous DRAM run

    in_maps = []
    for core in range(N_CORES):
        lo, hi = core * BL, (core + 1) * BL
        m = dict(consts)
        sc_ = s_all[lo:hi].reshape(NT // 16, 16, 128, DM).transpose(0, 2, 1, 3)
        m["sall"] = np.ascontiguousarray(sc_.reshape((NT // 16) * 128, 16 * DM))
        m["q"] = np.ascontiguousarray(q[lo:hi])
        mc = mask[lo:hi, 0, :]  # [BL, NN] bool
        # maskct[0, t*128 + p] = -1e10 * mask[2t + p//64, p%64]
        mct = (-1e10 * mc.reshape(NT, 128).astype(np.float32))
        m["maskct"] = mct.reshape(1, NT * 128).astype(ml_dtypes.bfloat16)
        in_maps.append(m)
    return in_maps


def kernel(**inputs):
    if "nc" not in _CACHE:
        _CACHE["nc"] = _build()
    nc = _CACHE["nc"]
    in_maps = _host_prep(inputs)
    res = run_bass_kernel_spmd(nc, in_maps, core_ids=list(range(N_CORES)))
    z = np.zeros((B, 1, F), np.float32)
    attn = np.zeros((H * B, 1, NN), np.float32)
    for core in range(N_CORES):
        r = res.results[core]
        lo = core * BL
        z[lo:lo + BL, 0, :] = r["z"]
        for h in range(H):
            attn[h * B + lo: h * B + lo + BL, 0, :] = r["attn"][h]
    return z, attn


# revision 27
# speedup vs baseline: 1.1306x; 1.0170x over previous
"""CAWN attention+merge kernel for Trainium2, 8-core data parallel.

Math notes (vs the reference):
  - NS=1, so softmax over n of (score_q + score_k) == softmax(score_k):
    the per-query score is a constant shift and cancels. wq is never needed.
  - kh/vh are never materialized:
      score_k[b,n,h] = k[b,n,:] @ wkvT[:,h],  wkvT[d,h] = sum_e wk[h*DK+e,d]*wm[DK+e]
      out_fc[b]      = sum_h ctx[b,h,:] @ W2T[h],  W2T[h] = (fc_w[:,hblk] @ wv[hblk]).T
      ctx[b,h,:]     = attn[b,h,:] @ k[b]
  - LayerNorm gamma/beta are folded into m1_w/m1_b (LN output only feeds m1).
Sharding: batch 1024 -> 8 cores x 128. Each core processes 64 "tiles" of
[128 rows = 2 batches x 64 neighbors, 512 features].
"""

import numpy as np
import ml_dtypes

import concourse.bass as bass
import concourse.bacc as bacc
import concourse.tile as tile
import concourse.mybir as mybir
from concourse.bass_utils import run_bass_kernel_spmd

N_CORES = 8
B, NN, F = 1024, 64, 128
DM = 4 * F            # 512
H, DK = 2, 256
BL = B // N_CORES     # 128 batch per core
NT = BL // 2          # 64 tiles per core
NCH = DM // 128       # 4 d-chunks
LN_EPS = 1e-5

f32 = mybir.dt.float32
bf16 = mybir.dt.bfloat16
MUL = mybir.AluOpType.mult
ADD = mybir.AluOpType.add
AF = mybir.ActivationFunctionType

_CACHE = {}


def _build():
    nc = bacc.Bacc("TRN2", target_bir_lowering=False)

    # ---- DRAM tensors (per-core shapes) ----
    s_all = nc.dram_tensor("sall", [(NT // 16) * 128, 16 * DM], bf16, kind="ExternalInput")
    q_in = nc.dram_tensor("q", [BL, DM], f32, kind="ExternalInput")
    maskc_in = nc.dram_tensor("maskc", [128, NT], f32, kind="ExternalInput")
    wkvt_in = nc.dram_tensor("wkvt", [128, NCH * H], bf16, kind="ExternalInput")
    w2t_in = nc.dram_tensor("w2t", [128, H * NCH * DM], bf16, kind="ExternalInput")
    m1t_in = nc.dram_tensor("m1t", [128, 5 * F], f32, kind="ExternalInput")
    m2t_in = nc.dram_tensor("m2t", [128, F], f32, kind="ExternalInput")
    fcb_in = nc.dram_tensor("fcb", [1, DM], f32, kind="ExternalInput")
    m1b_in = nc.dram_tensor("m1b", [128, 1], f32, kind="ExternalInput")
    m2b_in = nc.dram_tensor("m2b", [128, 1], f32, kind="ExternalInput")
    eps_in = nc.dram_tensor("eps", [128, 1], f32, kind="ExternalInput")
    identb_in = nc.dram_tensor("identb", [128, 128], bf16, kind="ExternalInput")
    identf_in = nc.dram_tensor("identf", [128, 128], f32, kind="ExternalInput")
    blk4_in = nc.dram_tensor("blk4", [128, 4], f32, kind="ExternalInput")
    blkblk_in = nc.dram_tensor("blkblk", [128, 128], bf16, kind="ExternalInput")

    z_out = nc.dram_tensor("z", [BL, F], f32, kind="ExternalOutput")
    attn_out = nc.dram_tensor("attn", [H, BL, NN], f32, kind="ExternalOutput")

    with tile.TileContext(nc) as tc:
        with (
            tc.tile_pool(name="consts", bufs=1) as consts,
            tc.tile_pool(name="kb", bufs=3) as kbp,
            tc.tile_pool(name="kt", bufs=6) as ktp_sb,
            tc.tile_pool(name="small", bufs=4) as small,
            tc.tile_pool(name="ctxs", bufs=2) as ctxsp,
            tc.tile_pool(name="tail", bufs=1) as tailp,
            tc.tile_pool(name="ps", bufs=3, space="PSUM") as ps,
            tc.tile_pool(name="ps2", bufs=2, space="PSUM") as ps2,
            tc.tile_pool(name="psacc", bufs=1, space="PSUM") as psacc,
        ):
            # ---- load constants ----
            def cload(tag, dram, shape, dtype):
                t = consts.tile(shape, dtype, tag=tag)
                nc.sync.dma_start(out=t[:], in_=dram[:])
                return t

            # critical-path constants first (needed by the streaming loop)
            identb = cload("identb", identb_in, [128, 128], bf16)
            wkvt = cload("wkvt", wkvt_in, [128, NCH * H], bf16)   # [p, c*2+h]
            maskc = cload("maskc", maskc_in, [128, NT], f32)
            blkblk = cload("blkblk", blkblk_in, [128, 128], bf16)
            blk4 = cload("blk4", blk4_in, [128, 4], f32)
            # late-use constants (g matmul + tail) loaded afterwards
            w2t = cload("w2t", w2t_in, [128, H * NCH * DM], bf16)
            q_sb = cload("q", q_in, [128, DM], f32)
            m1t = cload("m1t", m1t_in, [128, 5 * F], f32)
            m2t = cload("m2t", m2t_in, [128, F], f32)
            m1b = cload("m1b", m1b_in, [128, 1], f32)
            m2b = cload("m2b", m2b_in, [128, 1], f32)
            eps_sb = cload("eps", eps_in, [128, 1], f32)
            identf = cload("identf", identf_in, [128, 128], f32)

            fcb_rep = consts.tile([128, DM], f32, tag="fcb_rep")
            nc.sync.dma_start(
                out=fcb_rep[:],
                in_=bass.AP(tensor=fcb_in[:].tensor, offset=0,
                            ap=[[0, 128], [1, DM]]),
            )

            gp = psacc.tile([128, DM], f32, tag="g")

            ST = 16   # tiles per super-tile load
            KG = 16   # tiles per softmax batch group
            for half in range(2):
                ctxp = ps2.tile([128, NCH, 128], f32, tag="ctx")
                for jg in range(NT // 2 // KG):   # 4 groups of 8 tiles
                    tg0 = half * (NT // 2) + jg * KG
                    ti0 = jg * KG
                    # -- load K super-tile every ST tiles (pre-shuffled layout,
                    #    4 quarter-DMAs so consumers start early) --
                    if ti0 % ST == 0:
                        st_idx = (half * (NT // 2) + jg * KG) // ST
                        kbs = kbp.tile([128, ST, DM], bf16, tag="kbs")
                        for qq in range(4):
                            nc.gpsimd.dma_start(
                                out=kbs[:, qq * 4:(qq + 1) * 4, :],
                                in_=s_all[st_idx * 128:(st_idx + 1) * 128,
                                          qq * 4 * DM:(qq + 1) * 4 * DM],
                            )
                        attn_acc = small.tile([128, ST, 2], f32, tag="attn_acc")

                    scb = ps2.tile([128, 2 * KG * H], f32, tag="b")
                    sc = scb[:, 0:KG * H]
                    for jp in range(KG // 2):     # pairs of tiles
                        # transpose 8 chunks (2 tiles) into one psum bank
                        ktp = ps.tile([128, 2 * NCH, 128], bf16, tag="a")
                        kt = ktp_sb.tile([128, 2 * NCH, 128], bf16, tag="kt")
                        for u in range(2):
                            j = (ti0 % ST) + 2 * jp + u
                            kb = [kbs[:, j, c * 128:(c + 1) * 128]
                                  for c in range(NCH)]
                            for c in range(NCH):
                                nc.tensor.transpose(
                                    ktp[:, u * NCH + c, :], kb[c], identb[:])
                        # evacuate: ACT takes 3 chunks, DVE takes 5
                        nc.scalar.copy(kt[:, 0:3, :], ktp[:, 0:3, :])
                        nc.vector.tensor_copy(out=kt[:, 3:2 * NCH, :],
                                              in_=ktp[:, 3:2 * NCH, :])
                        # score (+ mask bias as a K=1 matmul)
                        for u in range(2):
                            j = 2 * jp + u
                            t = tg0 + j
                            for c in range(NCH):
                                nc.tensor.matmul(
                                    sc[:, 2 * j:2 * j + 2], kt[:, u * NCH + c, :],
                                    wkvt[:, c * H:(c + 1) * H],
                                    start=(c == 0), stop=(c == NCH - 1),
                                )

                    # -- batched softmax tail over KG tiles --
                    # mask add (broadcast over h), psum -> sbuf
                    masked = small.tile([128, KG * H], f32, tag="masked")
                    mk = maskc[:]
                    mrep = bass.AP(
                        tensor=mk.tensor, offset=mk.offset + tg0,
                        ap=[mk.ap[0], [1, KG], [0, 2]],
                    )
                    nc.vector.tensor_tensor(out=masked[:], in0=sc, in1=mrep, op=ADD)
                    expb = small.tile([128, KG * H], bf16, tag="expb")
                    nc.scalar.activation(out=expb[:], in_=masked[:], func=AF.Exp,
                                         bias=0.0, scale=1.0)
                    den = scb[:, KG * H:2 * KG * H]
                    nc.tensor.matmul(den, blkblk[:], expb[:], start=True, stop=True)
                    rden = small.tile([128, KG * H], f32, tag="rden")
                    nc.vector.reciprocal(rden[:], den)
                    nc.vector.tensor_tensor(
                        out=attn_acc[:, ti0 % ST:ti0 % ST + KG, :],
                        in0=expb[:], in1=rden[:], op=MUL,
                    )
                    # blocked attn for ctx matmuls: [128, (j,g,h)] bf16
                    attn4 = small.tile([128, KG * 4], bf16, tag="attn4")
                    af = attn_acc[:]
                    rep = bass.AP(
                        tensor=af.tensor, offset=af.offset + (ti0 % ST) * 2,
                        ap=[af.ap[0], [2, KG], [0, 2], [1, 2]],
                    )
                    blkr = bass.AP(
                        tensor=blk4[:].tensor, offset=blk4[:].offset,
                        ap=[blk4[:].ap[0], [0, KG], [1, 4]],
                    )
                    nc.vector.tensor_tensor(out=attn4[:], in0=rep, in1=blkr, op=MUL)
                    # ctx accumulation: ctxT[d, (tile,g,h)]
                    for j in range(KG):
                        ti = ti0 + j
                        kb = [kbs[:, ti % ST, c * 128:(c + 1) * 128]
                              for c in range(NCH)]
                        for c in range(NCH):
                            nc.tensor.matmul(
                                ctxp[:, c, 4 * ti:4 * ti + 4],
                                kb[c], attn4[:, 4 * j:4 * j + 4],
                                start=True, stop=True,
                            )

                    # -- attn output DMA once per super-tile (one per head) --
                    # dram elem addr = h*(BL*NN) + (2(t0+j)+g)*NN + n
                    if ti0 % ST == ST - KG:
                        t0 = tg0 - (ST - KG)
                        for h in range(H):
                            nc.sync.dma_start(
                                out=bass.AP(
                                    tensor=attn_out[:].tensor,
                                    offset=h * BL * NN + 2 * t0 * NN,
                                    ap=[[1, 128], [2 * NN, ST]],
                                ),
                                in_=attn_acc[:, :, h],
                            )

                # -- evacuate ctx half and run g matmuls --
                ctxs = ctxsp.tile([128, NCH, 128], bf16, tag="ctxs")
                nc.vector.tensor_copy(out=ctxs[:], in_=ctxp[:])
                for c in range(NCH):
                    for h in range(H):
                        lhs = bass.AP(
                            tensor=ctxs[:].tensor, offset=ctxs[:].offset + c * 128 + h,
                            ap=[ctxs[:].ap[0], [2, 64]],
                        )
                        nc.tensor.matmul(
                            gp[64 * half:64 * (half + 1), :],
                            lhs, w2t[:, (h * NCH + c) * DM:(h * NCH + c + 1) * DM],
                            start=(c == 0 and h == 0),
                            stop=(c == NCH - 1 and h == H - 1),
                        )

            # ---- tail: LeakyReLU(g+fcb) + q, LN, merge MLP ----
            g2 = tailp.tile([128, DM], f32)
            nc.vector.tensor_tensor(out=g2[:], in0=gp[:], in1=fcb_rep[:], op=ADD)
            lr = tailp.tile([128, DM], f32)
            nc.scalar.activation(out=lr[:], in_=g2[:], func=AF.Lrelu,
                                 bias=0.0, scale=1.0, alpha=0.2)
            xres = tailp.tile([128, DM], f32)
            nc.vector.tensor_tensor(out=xres[:], in0=lr[:], in1=q_sb[:], op=ADD)

            stats = tailp.tile([128, 6], f32)
            nc.vector.bn_stats(out=stats[:], in_=xres[:])
            mv = tailp.tile([128, 2], f32)
            nc.vector.bn_aggr(out=mv[:], in_=stats[:])
            sd = tailp.tile([128, 1], f32)
            nc.scalar.activation(out=sd[:], in_=mv[:, 1:2], func=AF.Sqrt,
                                 bias=eps_sb[:, 0:1], scale=1.0)
            rstd = tailp.tile([128, 1], f32)
            nc.vector.reciprocal(rstd[:], sd[:])
            negmr = tailp.tile([128, 1], f32)
            nc.vector.tensor_scalar(
                out=negmr[:], in0=mv[:, 0:1], scalar1=rstd[:, 0:1], scalar2=-1.0,
                op0=MUL, op1=MUL,
            )
            xn = tailp.tile([128, DM], f32)
            nc.scalar.activation(out=xn[:], in_=xres[:], func=AF.Identity,
                                 bias=negmr[:, 0:1], scale=rstd[:, 0:1])

            # x = [xn | src] transposed via PE (5 chunks of 128)
            xtp1 = ps.tile([128, 2, 128], f32, tag="a")
            xtp2 = ps2.tile([128, 2, 128], f32, tag="b")
            xtp3 = ps2.tile([128, 1, 128], f32, tag="ctx")
            for c in range(2):
                nc.tensor.transpose(xtp1[:, c, :], xn[:, c * 128:(c + 1) * 128], identf[:])
            for c in range(2):
                nc.tensor.transpose(xtp2[:, c, :], xn[:, (2 + c) * 128:(3 + c) * 128], identf[:])
            nc.tensor.transpose(xtp3[:, 0, :], q_sb[:, 0:F], identf[:])
            xts = tailp.tile([128, 5, 128], f32)
            nc.vector.tensor_copy(out=xts[:, 0:2, :], in_=xtp1[:])
            nc.vector.tensor_copy(out=xts[:, 2:4, :], in_=xtp2[:])
            nc.scalar.copy(xts[:, 4, :], xtp3[:, 0, :])

            hdnp = ps.tile([128, F], f32, tag="a")
            for c in range(5):
                nc.tensor.matmul(hdnp[:], m1t[:, c * F:(c + 1) * F], xts[:, c, :],
                                 start=(c == 0), stop=(c == 4))
            hdn = tailp.tile([128, F], f32)
            nc.scalar.activation(out=hdn[:], in_=hdnp[:], func=AF.Relu,
                                 bias=m1b[:, 0:1], scale=1.0)
            ztp = ps2.tile([128, F], f32, tag="b")
            nc.tensor.matmul(ztp[:], m2t[:], hdn[:], start=True, stop=True)
            zts = tailp.tile([128, F], f32)
            nc.scalar.activation(out=zts[:], in_=ztp[:], func=AF.Identity,
                                 bias=m2b[:, 0:1], scale=1.0)
            zp = ps2.tile([128, F], f32, tag="ctx")
            nc.tensor.transpose(zp[:], zts[:], identf[:])
            zs = tailp.tile([128, F], f32)
            nc.vector.tensor_copy(out=zs[:], in_=zp[:])
            nc.sync.dma_start(out=z_out[:], in_=zs[:])

    nc.compile()
    return nc


def _host_prep(inputs):
    src = np.asarray(inputs["src"], np.float32)
    src_t = np.asarray(inputs["src_t"], np.float32)
    src_p = np.asarray(inputs["src_p"], np.float32)
    mask = np.asarray(inputs["mask"])
    wk = np.asarray(inputs["wk"], np.float32)
    wv = np.asarray(inputs["wv"], np.float32)
    wm = np.asarray(inputs["wm"], np.float32)
    fc_w = np.asarray(inputs["fc_w"], np.float32)
    fc_b = np.asarray(inputs["fc_b"], np.float32)
    ln_g = np.asarray(inputs["ln_g"], np.float32)
    ln_b = np.asarray(inputs["ln_b"], np.float32)
    m1_w = np.asarray(inputs["m1_w"], np.float32)
    m1_b = np.asarray(inputs["m1_b"], np.float32)
    m2_w = np.asarray(inputs["m2_w"], np.float32)
    m2_b = np.asarray(inputs["m2_b"], np.float32)

    # q = concat(src, 0, src_t, src_p)  [B, DM]
    q = np.concatenate(
        [src[:, 0], np.zeros_like(src[:, 0]), src_t[:, 0], src_p[:, 0]], axis=-1
    ).astype(np.float32)

    # wkvT[d, h]
    wkvT = np.einsum("hed,e->dh", wk.reshape(H, DK, DM), wm[DK:]).astype(np.float32)
    # packed [128, c*2+h]
    wkvt_pack = np.ascontiguousarray(
        wkvT.reshape(NCH, 128, H).transpose(1, 0, 2).reshape(128, NCH * H)
    ).astype(ml_dtypes.bfloat16)

    # W2T[h] = (fc_w[:, hblk] @ wv[hblk]).T  -> [d, dd]
    w2t_pack = np.zeros((128, H * NCH * DM), np.float32)
    for h in range(H):
        w2 = fc_w[:, h * DK:(h + 1) * DK] @ wv[h * DK:(h + 1) * DK]   # [dd, d]
        w2T = w2.T  # [d, dd]
        for c in range(NCH):
            w2t_pack[:, (h * NCH + c) * DM:(h * NCH + c + 1) * DM] = \
                w2T[c * 128:(c + 1) * 128, :]
    w2t_pack = w2t_pack.astype(ml_dtypes.bfloat16)

    # LN folding into m1
    m1_w_eff = m1_w.copy()
    m1_w_eff[:, :DM] = m1_w[:, :DM] * ln_g[None, :]
    m1b_eff = m1_b + m1_w[:, :DM] @ ln_b
    m1T = m1_w_eff.T  # [640, 128]
    m1t_pack = np.ascontiguousarray(
        m1T.reshape(5, 128, F).transpose(1, 0, 2).reshape(128, 5 * F)
    ).astype(np.float32)
    m2t_pack = np.ascontiguousarray(m2_w.T).astype(np.float32)

    consts = dict(
        fcb=fc_b.reshape(1, DM).astype(np.float32),
        m1b=m1b_eff.reshape(128, 1).astype(np.float32),
        m2b=m2_b.reshape(128, 1).astype(np.float32),
        eps=np.full((128, 1), LN_EPS, np.float32),
        identb=np.eye(128, dtype=ml_dtypes.bfloat16),
        identf=np.eye(128, dtype=np.float32),
        wkvt=wkvt_pack, w2t=w2t_pack, m1t=m1t_pack, m2t=m2t_pack,
    )
    blk4 = np.zeros((128, 4), np.float32)
    blk4[:64, 0:2] = 1.0
    blk4[64:, 2:4] = 1.0
    consts["blk4"] = blk4
    blkblk = np.zeros((128, 128), np.float32)
    blkblk[:64, :64] = 1.0
    blkblk[64:, 64:] = 1.0
    consts["blkblk"] = blkblk.astype(ml_dtypes.bfloat16)

    s_all = np.concatenate(
        [np.asarray(inputs["seq"], np.float32),
         np.asarray(inputs["seq_e"], np.float32),
         np.asarray(inputs["seq_t"], np.float32),
         np.asarray(inputs["seq_p"], np.float32)], axis=-1
    ).astype(ml_dtypes.bfloat16)  # [B, NN, DM]; device only consumes bf16 K
    # pre-shuffle to SBUF layout: [supertile, partition, tile-in-super, feat]
    # so each partition's super-tile block is one contiguous DRAM run

    in_maps = []
    for core in range(N_CORES):
        lo, hi = core * BL, (core + 1) * BL
        m = dict(consts)
        sc_ = s_all[lo:hi].reshape(NT // 16, 16, 128, DM).transpose(0, 2, 1, 3)
        m["sall"] = np.ascontiguousarray(sc_.reshape((NT // 16) * 128, 16 * DM))
        m["q"] = np.ascontiguousarray(q[lo:hi])
        mc = mask[lo:hi, 0, :]  # [BL, NN] bool
        # maskc[p, t] = -1e10 * mask[2t + p//64, p%64]
        mc_t = mc.reshape(NT, 2, NN).transpose(1, 2, 0).reshape(128, NT)
        m["maskc"] = (-1e10 * mc_t.astype(np.float32)).astype(np.float32)
        in_maps.append(m)
    return in_maps


def kernel(**inputs):
    if "nc" not in _CACHE:
        _CACHE["nc"] = _build()
    nc = _CACHE["nc"]
    in_maps = _host_prep(inputs)
    res = run_bass_kernel_spmd(nc, in_maps, core_ids=list(range(N_CORES)))
    z = np.zeros((B, 1, F), np.float32)
    attn = np.zeros((H * B, 1, NN), np.float32)
    for core in range(N_CORES):
        r = res.results[core]
        lo = core * BL
        z[lo:lo + BL, 0, :] = r["z"]
        for h in range(H):
            attn[h * B + lo: h * B + lo + BL, 0, :] = r["attn"][h]
    return z, attn


# revision 34
# speedup vs baseline: 1.1646x; 1.0300x over previous
"""CAWN attention+merge kernel for Trainium2, 8-core data parallel.

Math notes (vs the reference):
  - NS=1, so softmax over n of (score_q + score_k) == softmax(score_k):
    the per-query score is a constant shift and cancels. wq is never needed.
  - kh/vh are never materialized:
      score_k[b,n,h] = k[b,n,:] @ wkvT[:,h],  wkvT[d,h] = sum_e wk[h*DK+e,d]*wm[DK+e]
      out_fc[b]      = sum_h ctx[b,h,:] @ W2T[h],  W2T[h] = (fc_w[:,hblk] @ wv[hblk]).T
      ctx[b,h,:]     = attn[b,h,:] @ k[b]
  - LayerNorm gamma/beta are folded into m1_w/m1_b (LN output only feeds m1).
Sharding: batch 1024 -> 8 cores x 128. Each core processes 64 "tiles" of
[128 rows = 2 batches x 64 neighbors, 512 features].
"""

import numpy as np
import ml_dtypes

import concourse.bass as bass
import concourse.bacc as bacc
import concourse.tile as tile
import concourse.mybir as mybir
from concourse.bass_utils import run_bass_kernel_spmd

N_CORES = 8
B, NN, F = 1024, 64, 128
DM = 4 * F            # 512
H, DK = 2, 256
BL = B // N_CORES     # 128 batch per core
NT = BL // 2          # 64 tiles per core
NCH = DM // 128       # 4 d-chunks
LN_EPS = 1e-5

f32 = mybir.dt.float32
bf16 = mybir.dt.bfloat16
MUL = mybir.AluOpType.mult
ADD = mybir.AluOpType.add
AF = mybir.ActivationFunctionType

_CACHE = {}


def _build():
    nc = bacc.Bacc("TRN2", target_bir_lowering=False)

    # ---- DRAM tensors (per-core shapes) ----
    s_all = nc.dram_tensor("sall", [(NT // 16) * 128, 16 * DM], bf16, kind="ExternalInput")
    q_in = nc.dram_tensor("q", [BL, DM], f32, kind="ExternalInput")
    maskct_in = nc.dram_tensor("maskct", [1, NT * 128], bf16, kind="ExternalInput")
    ones2_in = nc.dram_tensor("ones2", [1, 2], bf16, kind="ExternalInput")
    wkvt_in = nc.dram_tensor("wkvt", [128, NCH * H], bf16, kind="ExternalInput")
    w2t_in = nc.dram_tensor("w2t", [128, H * NCH * DM], bf16, kind="ExternalInput")
    m1t_in = nc.dram_tensor("m1t", [128, 5 * F], f32, kind="ExternalInput")
    m2t_in = nc.dram_tensor("m2t", [128, F], f32, kind="ExternalInput")
    fcb_in = nc.dram_tensor("fcb", [1, DM], f32, kind="ExternalInput")
    m1b_in = nc.dram_tensor("m1b", [128, 1], f32, kind="ExternalInput")
    m2b_in = nc.dram_tensor("m2b", [128, 1], f32, kind="ExternalInput")
    eps_in = nc.dram_tensor("eps", [128, 1], f32, kind="ExternalInput")
    identb_in = nc.dram_tensor("identb", [128, 128], bf16, kind="ExternalInput")
    identf_in = nc.dram_tensor("identf", [128, 128], f32, kind="ExternalInput")
    blk4_in = nc.dram_tensor("blk4", [128, 4], f32, kind="ExternalInput")
    blkblk_in = nc.dram_tensor("blkblk", [128, 128], bf16, kind="ExternalInput")

    z_out = nc.dram_tensor("z", [BL, F], f32, kind="ExternalOutput")
    attn_out = nc.dram_tensor("attn", [H, BL, NN], f32, kind="ExternalOutput")

    with tile.TileContext(nc) as tc:
        with (
            tc.tile_pool(name="consts", bufs=1) as consts,
            tc.tile_pool(name="kb", bufs=3) as kbp,
            tc.tile_pool(name="kt", bufs=6) as ktp_sb,
            tc.tile_pool(name="small", bufs=4) as small,
            tc.tile_pool(name="ctxs", bufs=2) as ctxsp,
            tc.tile_pool(name="tail", bufs=1) as tailp,
            tc.tile_pool(name="ps", bufs=3, space="PSUM") as ps,
            tc.tile_pool(name="ps2", bufs=2, space="PSUM") as ps2,
            tc.tile_pool(name="psacc", bufs=1, space="PSUM") as psacc,
        ):
            # ---- load constants ----
            def cload(tag, dram, shape, dtype):
                t = consts.tile(shape, dtype, tag=tag)
                nc.sync.dma_start(out=t[:], in_=dram[:])
                return t

            # critical-path constants first (needed by the streaming loop)
            identb = cload("identb", identb_in, [128, 128], bf16)
            wkvt = cload("wkvt", wkvt_in, [128, NCH * H], bf16)   # [p, c*2+h]
            maskct = cload("maskct", maskct_in, [1, NT * 128], bf16)
            ones2 = cload("ones2", ones2_in, [1, 2], bf16)
            blkblk = cload("blkblk", blkblk_in, [128, 128], bf16)
            blk4 = cload("blk4", blk4_in, [128, 4], f32)
            # late-use constants (g matmul + tail) loaded afterwards
            w2t = cload("w2t", w2t_in, [128, H * NCH * DM], bf16)
            q_sb = cload("q", q_in, [128, DM], f32)
            m1t = cload("m1t", m1t_in, [128, 5 * F], f32)
            m2t = cload("m2t", m2t_in, [128, F], f32)
            m1b = cload("m1b", m1b_in, [128, 1], f32)
            m2b = cload("m2b", m2b_in, [128, 1], f32)
            eps_sb = cload("eps", eps_in, [128, 1], f32)
            identf = cload("identf", identf_in, [128, 128], f32)

            fcb_rep = consts.tile([128, DM], f32, tag="fcb_rep")
            nc.sync.dma_start(
                out=fcb_rep[:],
                in_=bass.AP(tensor=fcb_in[:].tensor, offset=0,
                            ap=[[0, 128], [1, DM]]),
            )

            gp = psacc.tile([128, DM], f32, tag="g")

            ST = 16   # tiles per super-tile load == group == b-quarter
            KG = 16
            for jg in range(NT // KG):            # 4 groups of 16 tiles
                    tg0 = jg * KG
                    ti0 = 0
                    ctxp = ps2.tile([128, NCH, 64], f32, tag="ctx")
                    # -- load K super-tile (pre-shuffled layout, 4 quarter-DMAs) --
                    kbs = kbp.tile([128, ST, DM], bf16, tag="kbs")
                    for qq in range(4):
                        nc.gpsimd.dma_start(
                            out=kbs[:, qq * 4:(qq + 1) * 4, :],
                            in_=s_all[jg * 128:(jg + 1) * 128,
                                      qq * 4 * DM:(qq + 1) * 4 * DM],
                        )
                    attn_acc = small.tile([128, ST, 2], f32, tag="attn_acc")

                    scb = ps2.tile([128, 2 * KG * H], f32, tag="b")
                    sc = scb[:, 0:KG * H]
                    for jp in range(KG // 2):     # pairs of tiles
                        # transpose 8 chunks (2 tiles) into one psum bank
                        ktp = ps.tile([128, 2 * NCH, 128], bf16, tag="a")
                        kt = ktp_sb.tile([128, 2 * NCH, 128], bf16, tag="kt")
                        for u in range(2):
                            j = (ti0 % ST) + 2 * jp + u
                            kb = [kbs[:, j, c * 128:(c + 1) * 128]
                                  for c in range(NCH)]
                            for c in range(NCH):
                                nc.tensor.transpose(
                                    ktp[:, u * NCH + c, :], kb[c], identb[:])
                        # evacuate: ACT takes 3 chunks, DVE takes 5
                        nc.scalar.copy(kt[:, 0:3, :], ktp[:, 0:3, :])
                        nc.vector.tensor_copy(out=kt[:, 3:2 * NCH, :],
                                              in_=ktp[:, 3:2 * NCH, :])
                        # score (+ mask bias as a K=1 matmul)
                        for u in range(2):
                            j = 2 * jp + u
                            t = tg0 + j
                            for c in range(NCH):
                                nc.tensor.matmul(
                                    sc[:, 2 * j:2 * j + 2], kt[:, u * NCH + c, :],
                                    wkvt[:, c * H:(c + 1) * H],
                                    start=(c == 0), stop=False,
                                )
                            nc.tensor.matmul(
                                sc[:, 2 * j:2 * j + 2],
                                maskct[0:1, t * 128:(t + 1) * 128], ones2[0:1, :],
                                start=False, stop=True,
                            )

                    # -- batched softmax tail over KG tiles --
                    expb = small.tile([128, KG * H], bf16, tag="expb")
                    nc.scalar.activation(out=expb[:], in_=sc, func=AF.Exp,
                                         bias=0.0, scale=1.0)
                    den = scb[:, KG * H:2 * KG * H]
                    nc.tensor.matmul(den, blkblk[:], expb[:], start=True, stop=True)
                    rden = small.tile([128, KG * H], f32, tag="rden")
                    nc.vector.reciprocal(rden[:], den)
                    nc.vector.tensor_tensor(
                        out=attn_acc[:, ti0 % ST:ti0 % ST + KG, :],
                        in0=expb[:], in1=rden[:], op=MUL,
                    )
                    # blocked attn for ctx matmuls: [128, (j,g,h)] bf16
                    attn4 = small.tile([128, KG * 4], bf16, tag="attn4")
                    af = attn_acc[:]
                    rep = bass.AP(
                        tensor=af.tensor, offset=af.offset + (ti0 % ST) * 2,
                        ap=[af.ap[0], [2, KG], [0, 2], [1, 2]],
                    )
                    blkr = bass.AP(
                        tensor=blk4[:].tensor, offset=blk4[:].offset,
                        ap=[blk4[:].ap[0], [0, KG], [1, 4]],
                    )
                    nc.vector.tensor_tensor(out=attn4[:], in0=rep, in1=blkr, op=MUL)
                    # ctx accumulation: ctxT[d, (tile,g,h)] for this group
                    for j in range(KG):
                        kb = [kbs[:, j, c * 128:(c + 1) * 128]
                              for c in range(NCH)]
                        for c in range(NCH):
                            nc.tensor.matmul(
                                ctxp[:, c, 4 * j:4 * j + 4],
                                kb[c], attn4[:, 4 * j:4 * j + 4],
                                start=True, stop=True,
                            )

                    # -- attn output DMA (one per head per group) --
                    # dram elem addr = h*(BL*NN) + (2(t0+j)+g)*NN + n
                    for h in range(H):
                        nc.sync.dma_start(
                            out=bass.AP(
                                tensor=attn_out[:].tensor,
                                offset=h * BL * NN + 2 * tg0 * NN,
                                ap=[[1, 128], [2 * NN, ST]],
                            ),
                            in_=attn_acc[:, :, h],
                        )

                    # -- evacuate this group's ctx; g matmuls per group pair --
                    if jg % 2 == 0:
                        ctxs = ctxsp.tile([128, NCH, 128], bf16, tag="ctxs")
                    nc.vector.tensor_copy(
                        out=ctxs[:, :, 64 * (jg % 2):64 * (jg % 2) + 64],
                        in_=ctxp[:],
                    )
                    if jg % 2 == 1:
                        for c in range(NCH):
                            for h in range(H):
                                lhs = bass.AP(
                                    tensor=ctxs[:].tensor,
                                    offset=ctxs[:].offset + c * 128 + h,
                                    ap=[ctxs[:].ap[0], [2, 64]],
                                )
                                nc.tensor.matmul(
                                    gp[64 * (jg // 2):64 * (jg // 2 + 1), :],
                                    lhs,
                                    w2t[:, (h * NCH + c) * DM:(h * NCH + c + 1) * DM],
                                    start=(c == 0 and h == 0),
                                    stop=(c == NCH - 1 and h == H - 1),
                                )

            # ---- tail: LeakyReLU(g+fcb) + q, LN, merge MLP ----
            g2 = tailp.tile([128, DM], f32)
            nc.vector.tensor_tensor(out=g2[:], in0=gp[:], in1=fcb_rep[:], op=ADD)
            lr = tailp.tile([128, DM], f32)
            nc.scalar.activation(out=lr[:], in_=g2[:], func=AF.Lrelu,
                                 bias=0.0, scale=1.0, alpha=0.2)
            xres = tailp.tile([128, DM], f32)
            nc.vector.tensor_tensor(out=xres[:], in0=lr[:], in1=q_sb[:], op=ADD)

            stats = tailp.tile([128, 6], f32)
            nc.vector.bn_stats(out=stats[:], in_=xres[:])
            mv = tailp.tile([128, 2], f32)
            nc.vector.bn_aggr(out=mv[:], in_=stats[:])
            sd = tailp.tile([128, 1], f32)
            nc.scalar.activation(out=sd[:], in_=mv[:, 1:2], func=AF.Sqrt,
                                 bias=eps_sb[:, 0:1], scale=1.0)
            rstd = tailp.tile([128, 1], f32)
            nc.vector.reciprocal(rstd[:], sd[:])
            negmr = tailp.tile([128, 1], f32)
            nc.vector.tensor_scalar(
                out=negmr[:], in0=mv[:, 0:1], scalar1=rstd[:, 0:1], scalar2=-1.0,
                op0=MUL, op1=MUL,
            )
            xn = tailp.tile([128, DM], f32)
            nc.scalar.activation(out=xn[:], in_=xres[:], func=AF.Identity,
                                 bias=negmr[:, 0:1], scale=rstd[:, 0:1])

            # x = [xn | src] transposed via PE (5 chunks of 128)
            xtp1 = ps.tile([128, 2, 128], f32, tag="a")
            xtp2 = ps2.tile([128, 2, 128], f32, tag="b")
            xtp3 = ps2.tile([128, 1, 128], f32, tag="ctx")
            for c in range(2):
                nc.tensor.transpose(xtp1[:, c, :], xn[:, c * 128:(c + 1) * 128], identf[:])
            for c in range(2):
                nc.tensor.transpose(xtp2[:, c, :], xn[:, (2 + c) * 128:(3 + c) * 128], identf[:])
            nc.tensor.transpose(xtp3[:, 0, :], q_sb[:, 0:F], identf[:])
            xts = tailp.tile([128, 5, 128], f32)
            nc.vector.tensor_copy(out=xts[:, 0:2, :], in_=xtp1[:])
            nc.vector.tensor_copy(out=xts[:, 2:4, :], in_=xtp2[:])
            nc.scalar.copy(xts[:, 4, :], xtp3[:, 0, :])

            hdnp = ps.tile([128, F], f32, tag="a")
            for c in range(5):
                nc.tensor.matmul(hdnp[:], m1t[:, c * F:(c + 1) * F], xts[:, c, :],
                                 start=(c == 0), stop=(c == 4))
            hdn = tailp.tile([128, F], f32)
            nc.scalar.activation(out=hdn[:], in_=hdnp[:], func=AF.Relu,
                                 bias=m1b[:, 0:1], scale=1.0)
            ztp = ps2.tile([128, F], f32, tag="b")
            nc.tensor.matmul(ztp[:], m2t[:], hdn[:], start=True, stop=True)
            zts = tailp.tile([128, F], f32)
            nc.scalar.activation(out=zts[:], in_=ztp[:], func=AF.Identity,
                                 bias=m2b[:, 0:1], scale=1.0)
            zp = ps2.tile([128, F], f32, tag="ctx")
            nc.tensor.transpose(zp[:], zts[:], identf[:])
            zs = tailp.tile([128, F], f32)
            nc.vector.tensor_copy(out=zs[:], in_=zp[:])
            nc.sync.dma_start(out=z_out[:], in_=zs[:])

    nc.compile()
    return nc


def _host_prep(inputs):
    src = np.asarray(inputs["src"], np.float32)
    src_t = np.asarray(inputs["src_t"], np.float32)
    src_p = np.asarray(inputs["src_p"], np.float32)
    mask = np.asarray(inputs["mask"])
    wk = np.asarray(inputs["wk"], np.float32)
    wv = np.asarray(inputs["wv"], np.float32)
    wm = np.asarray(inputs["wm"], np.float32)
    fc_w = np.asarray(inputs["fc_w"], np.float32)
    fc_b = np.asarray(inputs["fc_b"], np.float32)
    ln_g = np.asarray(inputs["ln_g"], np.float32)
    ln_b = np.asarray(inputs["ln_b"], np.float32)
    m1_w = np.asarray(inputs["m1_w"], np.float32)
    m1_b = np.asarray(inputs["m1_b"], np.float32)
    m2_w = np.asarray(inputs["m2_w"], np.float32)
    m2_b = np.asarray(inputs["m2_b"], np.float32)

    # q = concat(src, 0, src_t, src_p)  [B, DM]
    q = np.concatenate(
        [src[:, 0], np.zeros_like(src[:, 0]), src_t[:, 0], src_p[:, 0]], axis=-1
    ).astype(np.float32)

    # wkvT[d, h]
    wkvT = np.einsum("hed,e->dh", wk.reshape(H, DK, DM), wm[DK:]).astype(np.float32)
    # packed [128, c*2+h]
    wkvt_pack = np.ascontiguousarray(
        wkvT.reshape(NCH, 128, H).transpose(1, 0, 2).reshape(128, NCH * H)
    ).astype(ml_dtypes.bfloat16)

    # W2T[h] = (fc_w[:, hblk] @ wv[hblk]).T  -> [d, dd]
    w2t_pack = np.zeros((128, H * NCH * DM), np.float32)
    for h in range(H):
        w2 = fc_w[:, h * DK:(h + 1) * DK] @ wv[h * DK:(h + 1) * DK]   # [dd, d]
        w2T = w2.T  # [d, dd]
        for c in range(NCH):
            w2t_pack[:, (h * NCH + c) * DM:(h * NCH + c + 1) * DM] = \
                w2T[c * 128:(c + 1) * 128, :]
    w2t_pack = w2t_pack.astype(ml_dtypes.bfloat16)

    # LN folding into m1
    m1_w_eff = m1_w.copy()
    m1_w_eff[:, :DM] = m1_w[:, :DM] * ln_g[None, :]
    m1b_eff = m1_b + m1_w[:, :DM] @ ln_b
    m1T = m1_w_eff.T  # [640, 128]
    m1t_pack = np.ascontiguousarray(
        m1T.reshape(5, 128, F).transpose(1, 0, 2).reshape(128, 5 * F)
    ).astype(np.float32)
    m2t_pack = np.ascontiguousarray(m2_w.T).astype(np.float32)

    consts = dict(
        fcb=fc_b.reshape(1, DM).astype(np.float32),
        m1b=m1b_eff.reshape(128, 1).astype(np.float32),
        m2b=m2_b.reshape(128, 1).astype(np.float32),
        eps=np.full((128, 1), LN_EPS, np.float32),
        identb=np.eye(128, dtype=ml_dtypes.bfloat16),
        identf=np.eye(128, dtype=np.float32),
        wkvt=wkvt_pack, w2t=w2t_pack, m1t=m1t_pack, m2t=m2t_pack,
    )
    blk4 = np.zeros((128, 4), np.float32)
    blk4[:64, 0:2] = 1.0
    blk4[64:, 2:4] = 1.0
    consts["blk4"] = blk4
    consts["ones2"] = np.ones((1, 2), ml_dtypes.bfloat16)
    blkblk = np.zeros((128, 128), np.float32)
    blkblk[:64, :64] = 1.0
    blkblk[64:, 64:] = 1.0
    consts["blkblk"] = blkblk.astype(ml_dtypes.bfloat16)

    s_all = np.concatenate(
        [np.asarray(inputs["seq"], np.float32),
         np.asarray(inputs["seq_e"], np.float32),
         np.asarray(inputs["seq_t"], np.float32),
         np.asarray(inputs["seq_p"], np.float32)], axis=-1
    ).astype(ml_dtypes.bfloat16)  # [B, NN, DM]; device only consumes bf16 K
    # pre-shuffle to SBUF layout: [supertile, partition, tile-in-super, feat]
    # so each partition's super-tile block is one contiguous DRAM run

    in_maps = []
    for core in range(N_CORES):
        lo, hi = core * BL, (core + 1) * BL
        m = dict(consts)
        sc_ = s_all[lo:hi].reshape(NT // 16, 16, 128, DM).transpose(0, 2, 1, 3)
        m["sall"] = np.ascontiguousarray(sc_.reshape((NT // 16) * 128, 16 * DM))
        m["q"] = np.ascontiguousarray(q[lo:hi])
        mc = mask[lo:hi, 0, :]  # [BL, NN] bool
        # maskct[0, t*128 + p] = -1e10 * mask[2t + p//64, p%64]
        mct = (-1e10 * mc.reshape(NT, 128).astype(np.float32))
        m["maskct"] = mct.reshape(1, NT * 128).astype(ml_dtypes.bfloat16)
        in_maps.append(m)
    return in_maps


def kernel(**inputs):
    if "nc" not in _CACHE:
        _CACHE["nc"] = _build()
    nc = _CACHE["nc"]
    in_maps = _host_prep(inputs)
    res = run_bass_kernel_spmd(nc, in_maps, core_ids=list(range(N_CORES)))
    z = np.zeros((B, 1, F), np.float32)
    attn = np.zeros((H * B, 1, NN), np.float32)
    for core in range(N_CORES):
        r = res.results[core]
        lo = core * BL
        z[lo:lo + BL, 0, :] = r["z"]
        for h in range(H):
            attn[h * B + lo: h * B + lo + BL, 0, :] = r["attn"][h]
    return z, attn
